# revision 17
# baseline (speedup 1.0000x reference)
"""Trainium2 Bass kernel for nn_CPDTail (CP-decomposed depthwise conv along H).

Computation:
    out[b,c,h,w] = sum_{k<3} sum_{r<8} x[b,h+k,c,r,w] * weight[c,r,k] + bias[c]
with x: (16,58,128,8,56) f32, weight: (128,8,3) f32, bias: (128,) f32,
out: (16,128,56,56) f32.

Sharding: data-parallel over batch B across the 8 NeuronCores (2 per core).

Per-core strategy (V4, default): the per-channel (r,k) contraction runs on the
TensorEngine as block-diagonal matmuls. Channels are processed in 4 groups of
32 with the rank dim split in halves; SBUF partitions hold p=(c_sub*4+r_low) so
the K=128 contraction covers (32 channels x 4 ranks). The stationary operand
lhsT[p, m<32] is block-diagonal (built host-side from `weight`), so
out[m, n] = sum_r w[c,r,k]*x[...] for the 32 channels of the group; the 2 rank
halves x 3 kernel taps accumulate in PSUM via h-shifted rhs slices
(tile_position pins each group's 32-aligned output strip). x is streamed from
HBM in hp-row blocks (host pre-transposed so every DMA is contiguous per
partition), cast fp32->bf16 on-chip (ScalarE+VectorE split), and matmul chunks
of 8 output rows (N=448 <= one PSUM bank) pipeline behind the stream. VectorE
evacuates PSUM with a fused +bias into SBUF; stores ride the ScalarE DGE ring
so the x stream owns the SyncE ring. Accumulation is fp32 in PSUM; measured
~97 us/core = slightly above the ~83 us HBM roofline for the 30 MB/core of
fp32 traffic. End-to-end l2 relative error ~2e-3 (bf16 input rounding).

Fallbacks (KERNEL_VARIANT env): v1 = exact-fp32 VectorE MAC (~257 us),
v2 = untuned PE pipeline, v3 = fp32 PE (exact, ~213 us), v4 = default.
"""

import os
import sys

if "/opt/trn_rl_repo" not in sys.path:
    sys.path.insert(0, "/opt/trn_rl_repo")

import ml_dtypes
import numpy as np

import concourse.bass as bass
import concourse.tile as tile
from concourse import bacc, mybir
from concourse import bass_utils

# Problem shape (hardcoded; kernel.py must be self-contained).
B, Hp, C, R, W = 16, 58, 128, 8, 56
KS = 3
H = Hp - 2  # 56 output rows (PAD=1, STRIDE=1)
NCORES = 8
BL = B // NCORES  # batches per core

G = 4          # channel groups
CG = C // G    # channels per group = 32
HC = 8         # output h rows per chunk
NCHUNK = H // HC  # 7 chunks

F32 = mybir.dt.float32
BF16 = mybir.dt.bfloat16
AL = mybir.AluOpType
ACT_COPY = mybir.ActivationFunctionType.Copy

VARIANT = os.environ.get("KERNEL_VARIANT", "v4")
STAGE_BUFS = int(os.environ.get("STAGE_BUFS", "3"))


def _new_nc():
    return bacc.Bacc("TRN2", target_bir_lowering=False, debug=False,
                     num_devices=NCORES)


# ---------------------------------------------------------------- V1 (fp32)
def _build_v1():
    """fp32 vector-engine kernel: 24 per-partition-scalar MAC terms."""
    nc = _new_nc()
    x_d = nc.dram_tensor("x", (BL, Hp, C, R, W), F32, kind="ExternalInput").ap()
    wb_d = nc.dram_tensor("wb", (C, R * KS + 1), F32, kind="ExternalInput").ap()
    o_d = nc.dram_tensor("out", (BL, C, H, W), F32, kind="ExternalOutput").ap()

    with tile.TileContext(nc) as tc:
        with (
            tc.tile_pool(name="consts", bufs=1) as consts,
            tc.tile_pool(name="xp", bufs=1) as xp,
            tc.tile_pool(name="accp", bufs=2) as accp,
        ):
            wb_sb = consts.tile([C, R * KS + 1], F32)
            nc.sync.dma_start(wb_sb[:], wb_d[:])

            for b in range(BL):
                # SBUF layout [c; hp, r, w] keeps DRAM-contiguous (r,w) runs.
                x_sb = xp.tile([C, Hp, R, W], F32, name=f"x_{b}", tag="x")
                nc.sync.dma_start(x_sb[:], x_d[b].rearrange("hp c r w -> c hp r w"))

                acc = accp.tile([C, H, W], F32, name=f"acc_{b}", tag="acc")
                first = True
                for r in range(R):
                    for k in range(KS):
                        xin = x_sb[:, k:k + H, r, :]
                        wsc = wb_sb[:, r * KS + k:r * KS + k + 1]
                        if first:
                            # acc = x*w + bias  (single-src, 2x mode)
                            nc.vector.tensor_scalar(
                                acc[:], xin, wsc, wb_sb[:, R * KS:],
                                AL.mult, AL.add)
                            first = False
                        else:
                            # acc = x*w + acc  (fused MAC)
                            nc.vector.scalar_tensor_tensor(
                                acc[:], xin, wsc, acc[:], AL.mult, AL.add)
                nc.sync.dma_start(o_d[b], acc[:])
    nc.compile()
    return nc


def _prep_v1(x, w, bias):
    wb = np.ascontiguousarray(
        np.concatenate([w.reshape(C, R * KS), bias.reshape(C, 1)], axis=1))
    return [{"x": x[c * BL:(c + 1) * BL], "wb": wb} for c in range(NCORES)]


# ------------------------------------------------- V2 (PE bf16 block-diagonal)
# hp staging blocks: chunk j needs hp rows [8j, 8j+10); block j ends at 8j+10.
_BLOCKS = [(0, 10)] + [(10 + 8 * i, 8) for i in range(6)]
NH = 2          # rank halves
RH = R // NH    # ranks per half = 4


def _build_v2():
    nc = _new_nc()
    x_d = nc.dram_tensor("x2", (BL, C, G, NH, Hp, W), F32,
                         kind="ExternalInput").ap()
    w_d = nc.dram_tensor("lhsT", (C, G, NH, KS, CG), BF16,
                         kind="ExternalInput").ap()
    b_d = nc.dram_tensor("bias", (C, 1), F32, kind="ExternalInput").ap()
    o_d = nc.dram_tensor("out", (BL, C, H, W), F32, kind="ExternalOutput").ap()

    with tile.TileContext(nc) as tc:
        with (
            tc.tile_pool(name="consts", bufs=1) as consts,
            tc.tile_pool(name="stage", bufs=3) as stage,
            tc.tile_pool(name="xbp", bufs=2) as xbp,
            tc.tile_pool(name="psum", bufs=4, space="PSUM") as psump,
            tc.tile_pool(name="outp", bufs=3) as outp,
        ):
            lhsT_sb = consts.tile([C, G, NH, KS, CG], BF16)
            nc.sync.dma_start(lhsT_sb[:], w_d[:])
            bias_sb = consts.tile([C, 1], F32)
            nc.sync.dma_start(bias_sb[:], b_d[:])

            for b in range(BL):
                # bf16 copy of this batch's x, partition p=(c_sub*4+r_low).
                xb = xbp.tile([C, G, NH, Hp, W], BF16, name=f"xb_{b}", tag="xb")
                for j, (r0, nr) in enumerate(_BLOCKS):
                    xs = stage.tile([C, G, NH, 10, W], F32, name=f"xs_{b}_{j}",
                                    tag="xs")
                    nc.sync.dma_start(xs[:, :, :, :nr, :],
                                      x_d[b, :, :, :, r0:r0 + nr, :])
                    # fp32 -> bf16 cast: split groups across ScalarE/VectorE.
                    nc.scalar.activation(xb[:, 0:2, :, r0:r0 + nr, :],
                                         xs[:, 0:2, :, :nr, :], ACT_COPY)
                    nc.vector.tensor_copy(xb[:, 2:4, :, r0:r0 + nr, :],
                                          xs[:, 2:4, :, :nr, :])

                    # chunk j is ready once block j is cast
                    h0 = HC * j
                    ps = psump.tile([C, HC * W], F32, name=f"ps_{b}_{j}",
                                    tag="ps")
                    for g in range(G):
                        nmm = 0
                        for hf in range(NH):
                            for k in range(KS):
                                nc.tensor.matmul(
                                    ps[CG * g:CG * (g + 1), :],
                                    lhsT_sb[:, g, hf, k, :],
                                    xb[:, g, hf, h0 + k:h0 + k + HC, :],
                                    start=(nmm == 0),
                                    stop=(nmm == NH * KS - 1),
                                    tile_position=(0, CG * g))
                                nmm += 1
                    ob = outp.tile([C, HC, W], F32, name=f"ob_{b}_{j}",
                                   tag="ob")
                    nc.vector.tensor_scalar(
                        ob[:].rearrange("c h w -> c (h w)"), ps[:],
                        bias_sb[:, 0:1], None, AL.add)
                    nc.sync.dma_start(o_d[b, :, h0:h0 + HC, :], ob[:])
    nc.compile()
    return nc


def _prep_v2(x, w, bias):
    # x2[b, cs*4+rh, g, hf, hp, w] = x[b, hp, 32g+cs, 4hf+rh, w]
    x2 = np.ascontiguousarray(
        x.reshape(B, Hp, G, CG, NH, RH, W).transpose(0, 3, 5, 2, 4, 1, 6)
        .reshape(B, C, G, NH, Hp, W))
    # lhsT[cs*4+rh, g, hf, k, m] = w[32g+m, 4hf+rh, k] if cs == m else 0
    wt = w.reshape(G, CG, NH, RH, KS)  # (g, cs, hf, rh, k)
    arr = np.zeros((CG, RH, G, NH, KS, CG), np.float32)
    for cs in range(CG):
        arr[cs, :, :, :, :, cs] = wt[:, cs, :, :, :].transpose(2, 0, 1, 3)
    lhsT = np.ascontiguousarray(
        arr.reshape(C, G, NH, KS, CG).astype(ml_dtypes.bfloat16))
    bias2 = np.ascontiguousarray(bias.reshape(C, 1))
    return [{"x2": x2[c * BL:(c + 1) * BL], "lhsT": lhsT, "bias": bias2}
            for c in range(NCORES)]


def _build_v3():
    """V2 structure but fp32 matmuls (rate experiment / exact path)."""
    nc = _new_nc()
    x_d = nc.dram_tensor("x2", (BL, C, G, NH, Hp, W), F32,
                         kind="ExternalInput").ap()
    w_d = nc.dram_tensor("lhsT", (C, G, NH, KS, CG), F32,
                         kind="ExternalInput").ap()
    b_d = nc.dram_tensor("bias", (C, 1), F32, kind="ExternalInput").ap()
    o_d = nc.dram_tensor("out", (BL, C, H, W), F32, kind="ExternalOutput").ap()

    with tile.TileContext(nc) as tc:
        with (
            tc.tile_pool(name="consts", bufs=1) as consts,
            tc.tile_pool(name="xbp", bufs=1) as xbp,
            tc.tile_pool(name="psum", bufs=4, space="PSUM") as psump,
            tc.tile_pool(name="outp", bufs=3) as outp,
        ):
            lhsT_sb = consts.tile([C, G, NH, KS, CG], F32)
            nc.sync.dma_start(lhsT_sb[:], w_d[:])
            bias_sb = consts.tile([C, 1], F32)
            nc.sync.dma_start(bias_sb[:], b_d[:])

            for b in range(BL):
                xb = xbp.tile([C, G, NH, Hp, W], F32, name=f"xb_{b}", tag="xb")
                for j, (r0, nr) in enumerate(_BLOCKS):
                    nc.sync.dma_start(xb[:, :, :, r0:r0 + nr, :],
                                      x_d[b, :, :, :, r0:r0 + nr, :])
                    h0 = HC * j
                    ps = psump.tile([C, HC * W], F32, name=f"ps_{b}_{j}",
                                    tag="ps")
                    for g in range(G):
                        nmm = 0
                        for hf in range(NH):
                            for k in range(KS):
                                nc.tensor.matmul(
                                    ps[CG * g:CG * (g + 1), :],
                                    lhsT_sb[:, g, hf, k, :],
                                    xb[:, g, hf, h0 + k:h0 + k + HC, :],
                                    start=(nmm == 0),
                                    stop=(nmm == NH * KS - 1),
                                    tile_position=(0, CG * g))
                                nmm += 1
                    ob = outp.tile([C, HC, W], F32, name=f"ob_{b}_{j}",
                                   tag="ob")
                    nc.vector.tensor_scalar(
                        ob[:].rearrange("c h w -> c (h w)"), ps[:],
                        bias_sb[:, 0:1], None, AL.add)
                    nc.sync.dma_start(o_d[b, :, h0:h0 + HC, :], ob[:])
    nc.compile()
    return nc


def _prep_v3(x, w, bias):
    x2 = np.ascontiguousarray(
        x.reshape(B, Hp, G, CG, NH, RH, W).transpose(0, 3, 5, 2, 4, 1, 6)
        .reshape(B, C, G, NH, Hp, W))
    wt = w.reshape(G, CG, NH, RH, KS)
    arr = np.zeros((CG, RH, G, NH, KS, CG), np.float32)
    for cs in range(CG):
        arr[cs, :, :, :, :, cs] = wt[:, cs, :, :, :].transpose(2, 0, 1, 3)
    lhsT = np.ascontiguousarray(arr.reshape(C, G, NH, KS, CG))
    bias2 = np.ascontiguousarray(bias.reshape(C, 1))
    return [{"x2": x2[c * BL:(c + 1) * BL], "lhsT": lhsT, "bias": bias2}
            for c in range(NCORES)]


# ---------------------------------------------------- V4 (tuned V2 pipeline)
# Variable-size output chunks so the PE can start after only 6 hp rows are
# resident, plus a small tail chunk. Chunk j consumes hp rows [h0, h0+hc+2).
_CHUNKS4 = [(0, 4)] + [(4 + 8 * i, 8) for i in range(6)] + [(52, 4)]
# Staging block j delivers exactly the extra hp rows chunk j needs.
_BLOCKS4 = [(0, 6), (6, 8), (14, 8), (22, 8), (30, 8), (38, 8), (46, 8),
            (54, 4)]


def _build_v4():
    nc = _new_nc()
    x_d = nc.dram_tensor("x2", (BL, C, G, NH, Hp, W), F32,
                         kind="ExternalInput").ap()
    w_d = nc.dram_tensor("lhsT", (C, G, NH, KS, CG), BF16,
                         kind="ExternalInput").ap()
    b_d = nc.dram_tensor("bias", (C, 1), F32, kind="ExternalInput").ap()
    o_d = nc.dram_tensor("out", (BL, C, H, W), F32, kind="ExternalOutput").ap()

    with tile.TileContext(nc) as tc:
        with (
            tc.tile_pool(name="consts", bufs=1) as consts,
            tc.tile_pool(name="stage", bufs=STAGE_BUFS) as stage,
            tc.tile_pool(name="xbp", bufs=2) as xbp,
            tc.tile_pool(name="psum", bufs=4, space="PSUM") as psump,
            tc.tile_pool(name="outp", bufs=3) as outp,
        ):
            # x stream owns the SP ring; constants ride the ACT HW-DGE ring
            # and are issued after the first x block so the stream leads.
            lhsT_sb = consts.tile([C, G, NH, KS, CG], BF16)
            bias_sb = consts.tile([C, 1], F32)
            consts_loaded = False

            for b in range(BL):
                xb = xbp.tile([C, G, NH, Hp, W], BF16, name=f"xb_{b}", tag="xb")
                for j, ((h0, hc), (r0, nr)) in enumerate(zip(_CHUNKS4,
                                                             _BLOCKS4)):
                    xs = stage.tile([C, G, NH, 8, W], F32, name=f"xs_{b}_{j}",
                                    tag="xs")
                    nc.sync.dma_start(xs[:, :, :, :nr, :],
                                      x_d[b, :, :, :, r0:r0 + nr, :])
                    if not consts_loaded:
                        nc.scalar.dma_start(lhsT_sb[:], w_d[:])
                        nc.scalar.dma_start(bias_sb[:], b_d[:])
                        consts_loaded = True
                    # fp32 -> bf16 cast: ~1/4 on ScalarE, 3/4 on VectorE.
                    nc.scalar.activation(xb[:, 0:1, :, r0:r0 + nr, :],
                                         xs[:, 0:1, :, :nr, :], ACT_COPY)
                    nc.vector.tensor_copy(xb[:, 1:4, :, r0:r0 + nr, :],
                                          xs[:, 1:4, :, :nr, :])

                    n = hc * W
                    ps = psump.tile([C, HC * W], F32, name=f"ps_{b}_{j}",
                                    tag="ps")
                    for g in range(G):
                        nmm = 0
                        for hf in range(NH):
                            for k in range(KS):
                                nc.tensor.matmul(
                                    ps[CG * g:CG * (g + 1), :n],
                                    lhsT_sb[:, g, hf, k, :],
                                    xb[:, g, hf, h0 + k:h0 + k + hc, :],
                                    start=(nmm == 0),
                                    stop=(nmm == NH * KS - 1),
                                    tile_position=(0, CG * g))
                                nmm += 1
                    ob = outp.tile([C, HC, W], F32, name=f"ob_{b}_{j}",
                                   tag="ob")
                    nc.vector.tensor_scalar(
                        ob[:].rearrange("c h w -> c (h w)")[:, :n], ps[:, :n],
                        bias_sb[:, 0:1], None, AL.add)
                    nc.scalar.dma_start(o_d[b, :, h0:h0 + hc, :],
                                        ob[:, :hc, :])
    nc.compile()
    return nc


_BUILDERS = {"v1": (_build_v1, _prep_v1), "v2": (_build_v2, _prep_v2),
             "v3": (_build_v3, _prep_v3), "v4": (_build_v4, _prep_v2)}
_NC_CACHE = {}


def _get_nc(variant):
    if variant not in _NC_CACHE:
        _NC_CACHE[variant] = _BUILDERS[variant][0]()
    return _NC_CACHE[variant]


def _run(inputs, trace=False, variant=None):
    variant = variant or VARIANT
    x = np.ascontiguousarray(np.asarray(inputs["x"], dtype=np.float32))
    w = np.ascontiguousarray(np.asarray(inputs["weight"], dtype=np.float32))
    bias = np.asarray(inputs["bias"], dtype=np.float32)
    assert x.shape == (B, Hp, C, R, W), x.shape

    nc = _get_nc(variant)
    in_maps = _BUILDERS[variant][1](x, w, bias)
    res = bass_utils.run_bass_kernel_spmd(
        nc, in_maps, core_ids=list(range(NCORES)), trace=trace)
    out = np.concatenate([r["out"] for r in res.results], axis=0)
    return out, res


def kernel(**inputs) -> np.ndarray:
    out, _ = _run(inputs, trace=False)
    return out
